# revision 46
# baseline (speedup 1.0000x reference)
"""Masked ragged-attention TRN2 kernel (nn_AttentionBase, B=16 Q=K=D=1024 fp32).

Sharding: data-parallel over batch, 2 batches per NeuronCore, 8 cores.
Per core (uniform SPMD program, masking driven purely by input data):
  scores = Q @ K^T          single-pass float32r matmuls (12-bit operands);
                            scores have sigma~32 so softmax rows are near
                            one-hot and tolerate ~1e-2 score noise
  scores += ones(q) x biasrow(k)   rank-1 matmul; biasrow = 0 / -1e30 per key
  softmax along k (free axis): negated reduce_max -> exp(x - max) on ScalarE
  with fused row-sum -> reciprocal; rows q >= query_len zeroed via the
  per-partition output scale
  out = softmax @ V         bf16 weights (PE-transposed) x bf16 V

Host packs Q^T/K^T pre-transposed AND pre-tiled so each SBUF tile is one
large DMA: qt[b][m][d][128][128], kt[b][n2][d][128][512], v[b][d][128][1024].
"""

import sys

sys.path.insert(0, "/opt/trn_rl_repo")

import numpy as np

P = 128
B_PER_CORE = 2
N_CORES = 8
SEQ = 1024
D = 1024
NCH = SEQ // P  # 8 chunks along any 1024 dim
NEG = np.float32(-1e30)

_CACHE = {}


def _build_nc():
    import concourse.bass as bass  # noqa: F401
    import concourse.mybir as mybir
    import concourse.tile as tile
    from concourse import bacc
    from concourse.masks import make_identity

    f32 = mybir.dt.float32
    f32r = mybir.dt.float32r
    bf16 = mybir.dt.bfloat16
    f8 = mybir.dt.float8e5
    X = mybir.AxisListType.X
    Exp = mybir.ActivationFunctionType.Exp
    DR = mybir.MatmulPerfMode.DoubleRow

    nc = bacc.Bacc("TRN2", target_bir_lowering=False, debug=False)
    q_dram = nc.dram_tensor(
        "qt", [B_PER_CORE, NCH, NCH, P, P], f32r, kind="ExternalInput"
    )  # [b, m, d, p, c]
    k_dram = nc.dram_tensor(
        "kt", [B_PER_CORE, 2, NCH, P, 512], f32r, kind="ExternalInput"
    )  # [b, n2, d, p, c]
    v_d = nc.dram_tensor("v", [B_PER_CORE, NCH, P, D], bf16, kind="ExternalInput")
    # fp8 bias pair-rows for the DoubleRow rank-1 bias matmul: row 0 holds
    # 0 / -32768 per key (|scores| <= ~183 so the bias can never win the row
    # max and exp(masked - max) underflows to exactly 0), row 1 is all zeros.
    bias_d = nc.dram_tensor("bias", [B_PER_CORE, 2, SEQ], f8, kind="ExternalInput")
    ones8_d = nc.dram_tensor("ones8", [1, 2, P], f8, kind="ExternalInput")
    qmask_d = nc.dram_tensor("qmask", [B_PER_CORE, SEQ], f32, kind="ExternalInput")
    out_d = nc.dram_tensor("out", [B_PER_CORE, SEQ, D], f32, kind="ExternalOutput")

    with tile.TileContext(nc) as tc:
        with (
            tc.tile_pool(name="const", bufs=1) as const_pool,
            tc.tile_pool(name="qk", bufs=2) as qk_pool,
            tc.tile_pool(name="v", bufs=2) as v_pool,
            tc.tile_pool(name="work", bufs=2) as work,
            tc.tile_pool(name="wpool", bufs=2) as wpool,
            tc.tile_pool(name="stat", bufs=6) as stat,
            tc.tile_pool(name="misc", bufs=2) as misc,
            tc.tile_pool(name="ps_s", bufs=3, space="PSUM") as ps_s,
            tc.tile_pool(name="ps_t", bufs=3, space="PSUM") as ps_t,
            tc.tile_pool(name="ps_o", bufs=1, space="PSUM") as ps_o,
        ):
            identity_f32 = const_pool.tile([P, P], f32, tag="ident32")
            make_identity(nc, identity_f32)
            identity = const_pool.tile([P, P], bf16, tag="ident")
            nc.vector.tensor_copy(identity[:], identity_f32[:])
            ones8 = const_pool.tile([1, 2, P], f8, tag="ones8")
            nc.gpsimd.dma_start(ones8[:], ones8_d.ap()[0:1, :, :])


            # p-state warmup: PE reaches full clock only after 3us of
            # continuous execution; burn junk transposes while the first
            # Q/K DMAs are in flight so real matmuls start at full speed.
            for _ in range(16):
                pwarm = ps_s.tile([P, 512], f32, tag="s", name="pwarm")
                nc.tensor.transpose(pwarm[:, 0:P], identity_f32[:], identity_f32[:])

            for b in range(B_PER_CORE):
                # SBUF tiles: per m a Q tile [P, d, P]; per (n2, half) a K
                # tile [P, 2, 512]. One DMA per tile; K is quartered so the
                # first QK chain starts after a 0.5MB transfer.
                qt_t = {
                    m: qk_pool.tile([P, NCH, P], f32r, tag=f"qt{m}", name=f"qt{m}")
                    for m in range(NCH)
                }
                kt_t = {}
                for n2 in range(2):
                    for h in range(4):
                        kt_t[(n2, h)] = qk_pool.tile(
                            [P, 2, 512], f32r, tag=f"kt{n2}{h}", name=f"kt{n2}{h}"
                        )

                def load_q(m):
                    # same queue (SP) as the K/V loads: transfers from one
                    # queue hit the DMA pool in program order, so the ramp
                    # consumes tiles in exactly the order issued below
                    nc.sync.dma_start(
                        qt_t[m][:], q_dram.ap()[b, m].rearrange("d p c -> p d c")
                    )

                def load_k(n2, h):
                    nc.sync.dma_start(
                        kt_t[(n2, h)][:],
                        k_dram.ap()[b, n2, h * 2 : (h + 1) * 2].rearrange(
                            "d p c -> p d c"
                        ),
                    )

                # ramp-critical order: everything the m0 score tile needs
                # first (incl. the tiny bias row, consumed at the END of the
                # first QK chain), then interleave V with the remaining Q
                # loads so the first PV chain isn't starved.
                load_q(0)
                load_k(0, 0)
                brow = misc.tile([1, 2, SEQ], f8, tag="brow")
                nc.gpsimd.dma_start(brow[:], bias_d.ap()[b : b + 1, :, :])
                qm = stat.tile([P, NCH], f32, tag="qm")
                nc.gpsimd.dma_start(qm[:], qmask_d.ap()[b].rearrange("(t p) -> p t", p=P))
                for h in range(1, 4):
                    load_k(0, h)
                load_q(1)
                for h in range(4):
                    load_k(1, h)
                # V before q2: the PE program order is QK0, QK1, PV0, QK2...
                # so the first PV chain consumes V right after QK1.
                vc = [None] * NCH
                for d in range(NCH):
                    t = v_pool.tile([P, D], bf16, tag=f"v{d}", name=f"v{d}")
                    nc.sync.dma_start(t[:], v_d.ap()[b, d])
                    vc[d] = t
                    if d == 3:
                        load_q(2)
                for m in range(3, NCH):
                    load_q(m)

                stageb = {}

                def emit_stage_a(m):
                    nm2 = stat.tile([P, 2], f32, tag="nm2", name="nm2")
                    negmax = stat.tile([P, 1], f32, tag="negmax", name="negmax")
                    w_sb = wpool.tile([P, SEQ], bf16, tag="w", name="w")
                    rs = stat.tile([P, 2], f32, tag="rs", name="rs")
                    pss = []
                    for n2 in range(2):
                        ps = ps_s.tile([P, 512], f32, tag="s", name=f"s{n2}")
                        for d in range(NCH):
                            nc.tensor.matmul(
                                ps[:],
                                qt_t[m][:, d],
                                kt_t[(n2, d // 2)][:, d % 2],
                                start=(d == 0),
                                stop=False,
                            )
                        nc.tensor.matmul(
                            ps[:],
                            ones8[:],
                            brow[:, :, n2 * 512 : (n2 + 1) * 512],
                            start=False,
                            stop=True,
                            perf_mode=DR,
                        )
                        nc.vector.reduce_max(
                            nm2[:, n2 : n2 + 1], ps[:], axis=X, negate=True
                        )
                        pss.append(ps)
                    nc.vector.tensor_tensor(
                        negmax[:], nm2[:, 0:1], nm2[:, 1:2], mybir.AluOpType.min
                    )
                    for n2 in range(2):
                        nc.scalar.activation(
                            w_sb[:, n2 * 512 : (n2 + 1) * 512],
                            pss[n2][:],
                            Exp,
                            bias=negmax[:],
                            accum_out=rs[:, n2 : n2 + 1],
                        )
                    rsum = stat.tile([P, 1], f32, tag="rsum", name="rsum")
                    nc.vector.tensor_tensor(
                        rsum[:], rs[:, 0:1], rs[:, 1:2], mybir.AluOpType.add
                    )
                    rcp = stat.tile([P, 1], f32, tag="rcp", name="rcp")
                    nc.vector.reciprocal(rcp[:], rsum[:])
                    scal = stat.tile([P, 1], f32, tag="scal", name="scal")
                    nc.vector.tensor_tensor(
                        scal[:], rcp[:], qm[:, m : m + 1], mybir.AluOpType.mult
                    )
                    stageb[m] = (w_sb, scal)

                def emit_stage_b(m):
                    w_sb, scal = stageb.pop(m)
                    wt = []
                    for j in range(NCH):
                        pst = ps_t.tile([P, P], bf16, tag="pst", name="pst")
                        nc.tensor.transpose(
                            pst[:], w_sb[:, j * P : (j + 1) * P], identity[:]
                        )
                        wtj = work.tile([P, P], bf16, tag=f"wt{j}", name=f"wt{j}")
                        nc.any.tensor_copy(wtj[:], pst[:])
                        wt.append(wtj)

                    out_sb = work.tile([P, D], f32, tag="outsb")
                    # store per n2-half so the first half's DMA overlaps the
                    # second half's PV chain; last tile goes via HWDGE (lower
                    # latency than SWDGE) to shorten the kernel-tail drain
                    last = b == B_PER_CORE - 1 and m == NCH - 1
                    out_eng = nc.sync if last else nc.gpsimd
                    for n2 in range(2):
                        po = ps_o.tile([P, 512], f32, tag=f"o{n2}", name=f"o{n2}")
                        for j in range(NCH):
                            nc.tensor.matmul(
                                po[:],
                                wt[j][:],
                                vc[j][:, n2 * 512 : (n2 + 1) * 512],
                                start=(j == 0),
                                stop=(j == NCH - 1),
                            )
                        nc.any.tensor_scalar_mul(
                            out_sb[:, n2 * 512 : (n2 + 1) * 512], po[:], scal[:]
                        )
                        out_eng.dma_start(
                            out_d.ap()[b, m * P : (m + 1) * P, n2 * 512 : (n2 + 1) * 512],
                            out_sb[:, n2 * 512 : (n2 + 1) * 512],
                        )

                for m in range(NCH + 1):
                    if m < NCH:
                        emit_stage_a(m)
                    if m >= 1:
                        emit_stage_b(m - 1)
    nc.compile()
    return nc


def _get_nc():
    if "nc" not in _CACHE:
        _CACHE["nc"] = _build_nc()
    return _CACHE["nc"]


def _q_layout(qT):
    """[d, q] transposed matrix -> [m, d, P, P] host layout."""
    # qt[m, d, p, c] = qT[d*P+p, m*P+c]
    return np.ascontiguousarray(qT.reshape(NCH, P, NCH, P).transpose(2, 0, 1, 3))


def _k_layout(kT):
    """[d, k] transposed matrix -> [n2, d, P, 512] host layout."""
    return np.ascontiguousarray(kT.reshape(NCH, P, 2, 512).transpose(2, 0, 1, 3))


def _prep_in_maps(queries, keys, values, query_lens, key_lens, order):
    """Build per-core input maps. order[c] = list of batch indices for core c."""
    import ml_dtypes

    bf16 = ml_dtypes.bfloat16
    f8 = ml_dtypes.float8_e5m2
    kidx = np.arange(SEQ)
    ones8 = np.zeros((1, 2, P), f8)
    ones8[0, 0, :] = f8(1.0)
    in_maps = []
    for c in range(N_CORES):
        bs = order[c]
        m = {
            "v": np.empty((B_PER_CORE, NCH, P, D), bf16),
            "bias": np.zeros((B_PER_CORE, 2, SEQ), f8),
            "ones8": ones8,
            "qmask": np.empty((B_PER_CORE, SEQ), np.float32),
            "qt": np.empty((B_PER_CORE, NCH, NCH, P, P), np.float32),
            "kt": np.empty((B_PER_CORE, 2, NCH, P, 512), np.float32),
        }
        for i, b in enumerate(bs):
            qT = np.ascontiguousarray(queries[b].T)
            kT = np.ascontiguousarray(keys[b].T)
            m["qt"][i] = _q_layout(qT)
            m["kt"][i] = _k_layout(kT)
            m["v"][i] = values[b].reshape(NCH, P, D).astype(bf16)
            m["bias"][i, 0] = np.where(kidx < key_lens[b], 0.0, -32768.0).astype(f8)
            m["qmask"][i] = (kidx < query_lens[b]).astype(np.float32)
        in_maps.append(m)
    return in_maps


def _run(inputs, trace=False, trace_kwargs=None):
    from concourse.bass_utils import run_bass_kernel_spmd

    queries = np.asarray(inputs["queries"], dtype=np.float32)
    keys = np.asarray(inputs["keys"], dtype=np.float32)
    values = np.asarray(inputs["values"], dtype=np.float32)
    query_lens = np.asarray(inputs["query_lens"]).astype(np.int64)
    key_lens = np.asarray(inputs["key_lens"]).astype(np.int64)
    B = queries.shape[0]
    assert B == N_CORES * B_PER_CORE

    order = [list(range(c * B_PER_CORE, (c + 1) * B_PER_CORE)) for c in range(N_CORES)]
    in_maps = _prep_in_maps(queries, keys, values, query_lens, key_lens, order)

    nc = _get_nc()
    kwargs = {}
    if trace:
        kwargs["trace"] = True
        if trace_kwargs:
            kwargs.update(trace_kwargs)
    try:
        res = run_bass_kernel_spmd(nc, in_maps, core_ids=list(range(N_CORES)), **kwargs)
    except Exception:
        # transient device wedges (NRT_EXEC_UNIT_UNRECOVERABLE) usually clear
        # on the next attempt
        import time

        time.sleep(5)
        res = run_bass_kernel_spmd(nc, in_maps, core_ids=list(range(N_CORES)), **kwargs)

    out = np.empty((B, SEQ, D), np.float32)
    for c in range(N_CORES):
        o = res.results[c]["out"]
        for i, b in enumerate(order[c]):
            out[b] = o[i]
    return out, res


def kernel(**inputs) -> np.ndarray:
    out, _ = _run(inputs, trace=False)
    return out
